# revision 24
# baseline (speedup 1.0000x reference)
"""Trainium2 Bass kernel for nn_Att_mlp_softmax (GNN message passing).

Reference computation:
    e = relu(h @ W1 + b1) @ W2 + b2                       # [N, 1] per-node score
    att = softmax(where(G > 0, e.T broadcast, -9e15))     # row-wise over neighbors
    out = (G.sum(-1))[:, None] * (att @ h)                # degree-rescaled aggregation

Because the pre-softmax score of entry (i, j) depends only on column j, the
masked softmax collapses algebraically:
    att[i, j] = G[i, j] * w[j] / sum_j G[i, j] * w[j],  w = exp(e + ESHIFT)
so with H' = [w * h | w | 1] (N x 130):
    Y = G @ H'
    out = Y[:, 129] * Y[:, :128] / Y[:, 128]
One big [N, N] x [N, 130] matmul replaces the N^2 softmax entirely.

Schedule (the original baseline ran the whole MLP ahead of the main loop in
the PE's in-order queue, so the main loop started ~17 us in and the PE sat
HAM-throttled at 1.2 GHz until 32 us):
  * The MLP and the main accumulation are interleaved per 512-node block with
    a 2-deep software pipeline -- PE queue order z0 z1 e0 [mm_b z_{b+2}
    e_{b+1}] -- so every PE op's producers (relu/exp/cast on scalar, hp on
    DVE) completed >= 1 block earlier and the PE never idles mid-loop.
  * PSUM accumulation groups are bank-granular (one pending group per 2 KB
    bank), so only 6 row-block accumulators coexist with the MLP's z bank +
    e/dummy bank.  Row blocks 6-7 run as a second pure-PE pass over the
    SBUF-resident hp/G tiles once the MLP banks free up; epilogues + output
    DMAs for blocks 0-5 overlap that pass.
  * DMA issues (~740 ns of issuing-engine time each) are spread across the
    sync / scalar / gpsimd queues so no queue serializes the critical path,
    and every transfer's source is its own fully-contiguous DRAM tensor.
  * G is fully prefetched into SBUF (64 KB/partition -- the whole 8 MB shard
    fits): first groups are 2 chunks so chunk 0 is ready early, later groups
    up to 16 chunks (2 MB) for cheap issues; DMA never idles behind
    consumption.
  * Warm-up dummy matmuls tied to the W1 DMA start the PE HAM clock-gate
    warm-up during the load phase.
  * Per-bank epilogue chains (tail copy -> recip -> scale) fire as each
    accumulator stops; output DMAs alternate sync/scalar queues.

Distribution: G is row-sharded across 8 NeuronCores (1024 rows each); h and
the MLP weights are replicated.  Each core's G shard is laid out
[128, JC, ROWS] (contraction-position major) so every DMA line is contiguous.
h is passed twice in bf16: d-major (hT, the MLP moving operand) and
chunk-major (hc, for the H' build).  The output is stored p-major
([128, 8, D]; host inverts).  No collectives.
"""

import numpy as np

N = 8192
D = 128
HID = 64
N_CORES = 8
ROWS = N // N_CORES          # 1024 output rows per core
JC = N // 128                # 64 contraction chunks of 128
NCOL = 130                   # H' columns: [w*h | w | 1]
ESHIFT = -1.0                # exp(e - 1): cancels exactly in the ratio
# G group sizes in chunks, round-robined across the three DMA rings in
# consumption order (gpsimd: G0, G1, G4; scalar: G2, G5, G8; sync: G3, G6,
# G7).  Small first groups land in time for chunk 0.  Sum must be JC.
G_GROUPS = [4, 4, 8, 8, 8, 8, 8, 8, 8]
# hc transfer sizes in chunks (first part early on scalar, rest on gpsimd)
HC_GROUPS = [8, 16, 16, 24]
ZB = 512                     # MLP z-block node count (pz = one full PSUM bank)
NZ = N // ZB                 # 16 z-blocks of 4 chunks each

_cache = {}


def _install_axon_hooks_shim():
    """Provide antenv.axon_hooks if the image lacks it (trn_boot step 6).

    concourse.bass_utils imports it unconditionally when BASS_TRACE is set;
    without the shim that import crashes instead of degrading.
    """
    import contextlib
    import ctypes
    import sys
    import types

    try:
        import antenv.axon_hooks  # noqa: F401
        return
    except ImportError:
        pass

    so_path = "/opt/axon/libaxon_pjrt.so"

    def _make_hook():
        try:
            lib = ctypes.CDLL(so_path)
        except OSError:
            return None
        if not hasattr(lib, "axon_start_nrt_profile"):
            return None
        lib.axon_start_nrt_profile.argtypes = [
            ctypes.POINTER(ctypes.c_int64),
            ctypes.c_size_t,
        ]
        lib.axon_start_nrt_profile.restype = ctypes.c_int64
        lib.axon_stop_nrt_profile.argtypes = [ctypes.c_char_p]
        lib.axon_stop_nrt_profile.restype = ctypes.c_int64

        @contextlib.contextmanager
        def _hook(output_dir, device_ids):
            import jax

            jax.devices()
            if device_ids:
                ids = (ctypes.c_int64 * len(device_ids))(*device_ids)
                rc = lib.axon_start_nrt_profile(ids, len(device_ids))
            else:
                rc = lib.axon_start_nrt_profile(None, 0)
            if rc != 0:
                raise RuntimeError(f"axon_start_nrt_profile rc={rc}")
            try:
                yield
            finally:
                lib.axon_stop_nrt_profile(str(output_dir).encode())

        return _hook

    mod = types.ModuleType("antenv.axon_hooks")
    _holder = {"hook": _make_hook()}
    mod.set_axon_ntff_profile_hook = lambda h: _holder.__setitem__("hook", h)
    mod.get_axon_ntff_profile_hook = lambda: _holder["hook"]
    sys.modules["antenv.axon_hooks"] = mod
    try:
        import antenv

        antenv.axon_hooks = mod
    except ImportError:
        pass


def build_nc(enable_asserts=False):
    """Build + compile the per-core Bass program (identical on all 8 cores)."""
    from concourse import bacc, mybir, tile

    f32 = mybir.dt.float32
    bf16 = mybir.dt.bfloat16
    f8 = mybir.dt.float8e4
    AF = mybir.ActivationFunctionType
    ALU = mybir.AluOpType

    nc = bacc.Bacc(
        "TRN2",
        target_bir_lowering=False,
        debug=False,
        enable_asserts=enable_asserts,
        num_devices=N_CORES,
    )
    # Every DMA source is its own fully-contiguous DRAM tensor (the host
    # pre-slices): with partition stride == line size the whole transfer is
    # one linear HBM stream, vs ~300 GB/s measured with 16-64 KB strides.
    g8 = [
        nc.dram_tensor(f"g8_{gi}", [128, gn, ROWS], f8, kind="ExternalInput").ap()
        for gi, gn in enumerate(G_GROUPS)
    ]
    hta0 = nc.dram_tensor("hta0", [D, 512], bf16, kind="ExternalInput").ap()
    hta1 = nc.dram_tensor("hta1", [D, 1536], bf16, kind="ExternalInput").ap()
    htb = nc.dram_tensor("htb", [D, 2048], bf16, kind="ExternalInput").ap()
    htc = nc.dram_tensor("htc", [D, 4096], bf16, kind="ExternalInput").ap()
    hcq = [
        nc.dram_tensor(f"hcq{q}", [128, hn, D], bf16, kind="ExternalInput").ap()
        for q, hn in enumerate(HC_GROUPS)
    ]
    W1 = nc.dram_tensor("W1", [D, HID], bf16, kind="ExternalInput").ap()
    W2 = nc.dram_tensor("W2", [HID, 1], bf16, kind="ExternalInput").ap()
    # cb packs [b2e | b1]: col 0 = b2 + ESHIFT broadcast (exp bias), col 1
    # rows 0-63 = b1 (relu bias) -- one tiny DMA instead of two
    cb = nc.dram_tensor("cb", [128, 2], f32, kind="ExternalInput").ap()
    out = nc.dram_tensor("out", [128, 8, D], f32, kind="ExternalOutput").ap()

    g_start = [sum(G_GROUPS[:i]) for i in range(len(G_GROUPS))]

    def glookup(jc):
        for gi in range(len(G_GROUPS) - 1, -1, -1):
            if jc >= g_start[gi]:
                return gi, jc - g_start[gi]
        raise AssertionError

    with tile.TileContext(nc) as tc:
        with (
            tc.tile_pool(name="const", bufs=1) as cpool,
            tc.tile_pool(name="big", bufs=1) as bigpool,
            tc.tile_pool(name="gbuf", bufs=len(G_GROUPS)) as gpool,
            tc.tile_pool(name="outbuf", bufs=1) as opool,
            tc.tile_pool(name="small", bufs=2) as spool,
            tc.tile_pool(name="ps_a6", bufs=6, space="PSUM") as ps_a6,
        ):
            # ---- DMA issue plan (emission order == per-engine queue order).
            # Each engine owns one FIFO DGE ring (per-transfer fixed cost
            # ~1.6-2 us, aggregate ~330 GB/s), so transfers are round-robined
            # across the three rings in CONSUMPTION order with balanced ring
            # loads (~4 MB each), criticals first:
            # sync:   W1, hta0, hta1, htb, G3, G6, G7, [outs]
            # scalar: hc0a, G2, htc, G5, G8, [outs]
            # gpsimd: cb, W2, G0, G1, hc0b, G4, hc1a, hc1b
            W1_sb = cpool.tile([D, HID], bf16)
            nc.sync.dma_start(W1_sb[:], W1[:])

            hta0_sb = bigpool.tile([D, 512], bf16, name="hta0")
            hta1_sb = bigpool.tile([D, 1536], bf16, name="hta1")
            htb_sb = bigpool.tile([D, 2048], bf16, name="htb")
            htc_sb = bigpool.tile([D, 4096], bf16, name="htc")

            def htile(b):
                # z-block b covers nodes [ZB*b, ZB*b+ZB)
                n0 = ZB * b
                if n0 < 512:
                    return hta0_sb, slice(n0, n0 + ZB)
                if n0 < 2048:
                    return hta1_sb, slice(n0 - 512, n0 - 512 + ZB)
                if n0 < 4096:
                    return htb_sb, slice(n0 - 2048, n0 - 2048 + ZB)
                return htc_sb, slice(n0 - 4096, n0 - 4096 + ZB)

            nc.sync.dma_start(hta0_sb[:], hta0[:])
            nc.sync.dma_start(hta1_sb[:], hta1[:])
            nc.sync.dma_start(htb_sb[:], htb[:])

            gt_sb = []
            for gi, gn in enumerate(G_GROUPS):
                gt_sb.append(gpool.tile([128, gn, ROWS], f8, tag="gt", name=f"gt{gi}"))

            hc_sb = bigpool.tile([128, JC, D], bf16)
            hc_start = [sum(HC_GROUPS[:i]) for i in range(len(HC_GROUPS))]

            def hc_issue(q, eng):
                cl = slice(hc_start[q], hc_start[q] + HC_GROUPS[q])
                eng.dma_start(hc_sb[:, cl, :], hcq[q][:])

            def g_issue(gi, eng):
                eng.dma_start(gt_sb[gi][:], g8[gi][:])

            g_issue(3, nc.sync)
            g_issue(6, nc.sync)
            g_issue(7, nc.sync)

            hc_issue(0, nc.scalar)
            g_issue(2, nc.scalar)
            nc.scalar.dma_start(htc_sb[:], htc[:])
            g_issue(5, nc.scalar)
            g_issue(8, nc.scalar)

            cb_sb = cpool.tile([128, 2], f32)
            nc.gpsimd.dma_start(cb_sb[:], cb[:])
            b2e_sb = cb_sb[:, 0:1]
            b1_sb = cb_sb[0:HID, 1:2]
            W2_sb = cpool.tile([HID, 1], bf16)
            nc.gpsimd.dma_start(W2_sb[:], W2[:])
            g_issue(0, nc.gpsimd)
            g_issue(1, nc.gpsimd)
            hc_issue(1, nc.gpsimd)
            g_issue(4, nc.gpsimd)
            hc_issue(2, nc.gpsimd)
            hc_issue(3, nc.gpsimd)

            # ---- SBUF working tensors
            a_sb = bigpool.tile([HID, N], bf16)     # relu(h @ W1 + b1)
            w_sb = cpool.tile([128, JC], f32)       # exp(e + ESHIFT)
            # hp_big holds every H' chunk: col 128 (w, bf16) is written by
            # the scalar cast, col 129 is memset to 1 once, cols 0-127 by
            # one DVE op per chunk
            hp_big = bigpool.tile([128, JC, NCOL], bf16, name="hp_big")
            warm = cpool.tile([128, 128], bf16)
            nc.vector.memset(warm[:], 0.0)
            nc.vector.memset(hp_big[:, :, 129], 1.0)

            # ---- PSUM layout: accumulation groups are BANK-granular (one
            # pending group per 2 KB bank).  6 row-block accumulators + the
            # MLP's z bank + e/dummy bank fill all 8; row blocks 6-7 run as
            # a second pure-PE pass over the SBUF-resident hp/G tiles once
            # the MLP banks free up.
            accs6 = [ps_a6.tile([128, NCOL], f32, tag="acc", name=f"acc{it}")
                     for it in range(6)]

            def build_hp(jc):
                # just-in-time H' chunk build: 1 DVE op (tail cols are
                # prefilled by the scalar cast + the ones memset)
                nc.vector.tensor_scalar_mul(
                    hp_big[:, jc, 0:128], hc_sb[:, jc, :],
                    w_sb[:, jc : jc + 1]
                )

            with tc.tile_pool(name="ps_mlp", bufs=1, space="PSUM") as ps_mlp:
                pe = ps_mlp.tile([128, 192], f32, tag="pe", bufs=1, name="pe")

                def dummy_mm(stat, n=1):
                    # HAM warm-keeping: tiny matmul gated only on stat's DMA
                    for _ in range(n):
                        nc.tensor.matmul(
                            pe[0:64, 64:192], stat, warm[:], start=True,
                            stop=True,
                        )

                dummy_mm(W1_sb[:, 0:64], 3)

                def mlp_z(b):
                    # z matmul + relu for block b's 512 nodes.  relu runs as
                    # a fused DVE add+max, shortening the single-buffered
                    # pz's z -> relu -> z WAR round-trip to ~1.4 us, under
                    # the 1.7 us block cadence.
                    pz = ps_mlp.tile([HID, ZB], f32, tag="pz", bufs=1)
                    t, hsl = htile(b)
                    nc.tensor.matmul(
                        pz[:], W1_sb[:], t[:, hsl], start=True, stop=True
                    )
                    sl = slice(b * ZB, (b + 1) * ZB)
                    nc.vector.tensor_scalar(
                        a_sb[:, sl], pz[:], b1_sb, 0.0,
                        ALU.add, ALU.max,
                    )

                def mlp_e(b):
                    # 4x e matmuls -> exp -> w cast for block b's chunks.
                    # The cast writes w straight into hp_big col 128.
                    for c in range(4 * b, 4 * b + 4):
                        nc.tensor.matmul(
                            pe[:, c : c + 1],
                            a_sb[:, c * 128 : (c + 1) * 128],
                            W2_sb[:, 0:1],
                            start=True,
                            stop=True,
                        )
                    ql = slice(4 * b, 4 * b + 4)
                    nc.scalar.activation(
                        w_sb[:, ql], pe[:, 4 * b : 4 * b + 4],
                        AF.Exp, bias=b2e_sb,
                    )
                    nc.scalar.activation(
                        hp_big[:, ql, 128], w_sb[:, ql], AF.Copy
                    )

                def mm_block(b):
                    for jc in range(4 * b, 4 * b + 4):
                        build_hp(jc)
                        gi, jci = glookup(jc)
                        for it in range(6):
                            nc.tensor.matmul(
                                accs6[it][:],
                                gt_sb[gi][:, jci, it * 128 : (it + 1) * 128],
                                hp_big[:, jc, :],
                                start=(jc == 0),
                                stop=(jc == JC - 1),
                            )

                # ---- pass 1: software-pipelined MLP + accumulation for row
                # blocks 0-5.  z runs 3 blocks ahead and e 2 ahead of the
                # accumulation (PE order z0 z1 z2 e0 e1 [mm_b z_{b+3}
                # e_{b+2}]) so the ~2.6 us PE->scalar->DVE->PE w-chain per
                # block stays hidden behind 2 blocks (~3.4 us) of slack.
                for b in range(3):
                    mlp_z(b)
                mlp_e(0)
                mlp_e(1)
                for b in range(NZ):
                    mm_block(b)
                    if b + 3 < NZ:
                        mlp_z(b + 3)
                    if b + 2 < NZ:
                        mlp_e(b + 2)

            with tc.tile_pool(name="ps_a2", bufs=2, space="PSUM") as ps_a2:
                # ---- pass 2: row blocks 6-7 over the resident hp/G tiles,
                # it-major so block 6's epilogue + output DMA overlap block
                # 7's sweep
                accs2 = [ps_a2.tile([128, NCOL], f32, tag="acc2",
                                    name=f"acc2_{i}") for i in range(2)]
                for i, it in enumerate((6, 7)):
                    for jc in range(JC):
                        gi, jci = glookup(jc)
                        nc.tensor.matmul(
                            accs2[i][:],
                            gt_sb[gi][:, jci, it * 128 : (it + 1) * 128],
                            hp_big[:, jc, :],
                            start=(jc == 0),
                            stop=(jc == JC - 1),
                        )

                # epilogue, fully per-bank: each bank's whole chain (tail copy
                # -> recip -> r -> scaled output) runs as soon as ITS
                # accumulator stops; row blocks 0-6 scale + store during pass
                # 2.  Output DMAs alternate sync/scalar queues.
                ot_all = opool.tile([128, 8, D], f32, tag="ot_all", bufs=1)
                for it in range(8):
                    acc = accs6[it] if it < 6 else accs2[it - 6]
                    tl = spool.tile([128, 2], f32, tag="tl", name=f"tl{it}",
                                    bufs=8)
                    nc.vector.tensor_copy(tl[:], acc[:, 128:130])
                    den = spool.tile([128, 1], f32, tag="den", name=f"den{it}",
                                     bufs=8)
                    nc.vector.tensor_scalar_add(den[:], tl[:, 0:1], 1e-30)
                    rc = spool.tile([128, 1], f32, tag="rc", name=f"rc{it}",
                                    bufs=8)
                    nc.vector.reciprocal(rc[:], den[:])
                    r1 = spool.tile([128, 1], f32, tag="r1", name=f"r1{it}",
                                    bufs=8)
                    nc.vector.tensor_mul(r1[:], rc[:], tl[:, 1:2])
                    nc.vector.tensor_scalar_mul(
                        ot_all[:, it, :], acc[:, 0:128], r1[:]
                    )
                    if it in (1, 3, 5):
                        eng = nc.sync if it in (1, 5) else nc.scalar
                        eng.dma_start(
                            out[:, it - 1 : it + 1, :],
                            ot_all[:, it - 1 : it + 1, :],
                        )
                    elif it == 6:
                        nc.scalar.dma_start(out[:, 6:7, :], ot_all[:, 6:7, :])
                    elif it == 7:
                        nc.sync.dma_start(out[:, 7:8, :], ot_all[:, 7:8, :])

    nc.compile()
    return nc


def make_in_maps(graph_info, h, W1, b1, W2, b2):
    """Shard + lay out the full inputs for the 8 cores.

    Every device tensor is a fully-contiguous DRAM block matching exactly one
    DMA transfer (per-group G, per-tile hT, per-part hc), so each transfer
    is one linear HBM stream.
    """
    import ml_dtypes

    bf16 = ml_dtypes.bfloat16
    f8 = ml_dtypes.float8_e4m3fn

    # G (exact 0/1) as fp8, laid out [core][128 c, JC, ROWS] so the stationary
    # tile for (chunk jc, row block it) is g8[:, jc, it*128:(it+1)*128]
    g = np.asarray(graph_info, np.float32)
    G8 = g.astype(f8).reshape(N_CORES, ROWS, JC, 128).transpose(0, 3, 2, 1)
    g_start = [sum(G_GROUPS[:i]) for i in range(len(G_GROUPS))]
    h = np.asarray(h, np.float32)
    hTb = np.ascontiguousarray(h.T).astype(bf16)               # [D, N]
    hcb = np.ascontiguousarray(
        h.reshape(JC, 128, D).transpose(1, 0, 2)               # [128, JC, D]
    ).astype(bf16)
    W1b = np.asarray(W1, np.float32).astype(bf16)
    W2r = np.asarray(W2, np.float32).reshape(HID, 1).astype(bf16)
    cb = np.zeros((128, 2), np.float32)
    cb[:, 0] = float(np.asarray(b2).reshape(())) + ESHIFT
    cb[:HID, 1] = np.asarray(b1, np.float32).reshape(HID)
    hc_start = [sum(HC_GROUPS[:i]) for i in range(len(HC_GROUPS))]

    common = {
        "W1": W1b,
        "W2": W2r,
        "cb": cb,
        "hta0": np.ascontiguousarray(hTb[:, 0:512]),
        "hta1": np.ascontiguousarray(hTb[:, 512:2048]),
        "htb": np.ascontiguousarray(hTb[:, 2048:4096]),
        "htc": np.ascontiguousarray(hTb[:, 4096:8192]),
    }
    for q, hn in enumerate(HC_GROUPS):
        common[f"hcq{q}"] = np.ascontiguousarray(
            hcb[:, hc_start[q] : hc_start[q] + hn, :]
        )

    in_maps = []
    for c in range(N_CORES):
        m = dict(common)
        for gi, gn in enumerate(G_GROUPS):
            m[f"g8_{gi}"] = np.ascontiguousarray(
                G8[c][:, g_start[gi] : g_start[gi] + gn, :]
            )
        in_maps.append(m)
    return in_maps


def kernel(graph_info, h, W1, b1, W2, b2):
    _install_axon_hooks_shim()
    from concourse.bass_utils import run_bass_kernel_spmd

    if "nc" not in _cache:
        _cache["nc"] = build_nc()
    nc = _cache["nc"]

    in_maps = make_in_maps(graph_info, h, W1, b1, W2, b2)
    res = run_bass_kernel_spmd(nc, in_maps, list(range(N_CORES)))
    # out is stored p-major [128, 8, D] per core; invert to row order
    return np.concatenate(
        [
            res.results[c]["out"].transpose(1, 0, 2).reshape(ROWS, D)
            for c in range(N_CORES)
        ],
        axis=0,
    )
